# revision 17
# baseline (speedup 1.0000x reference)
"""Causal multi-head attention block (qkv -> attention -> proj) on 8 TRN2 cores.

Problem: x[2,2048,1024], w_qkv[3072,1024], b_qkv[3072], w_proj[1024,1024],
b_proj[1024]; H=16 heads, D=64; softmax scale 1/sqrt(1024).

Sharding: core = (batch b, head-group hg); 2 batches x 4 groups of 4 heads.
Each core computes qkv for its 4 heads, causal attention, and a partial
projection (its heads' columns of w_proj); host sums the 4 partials per batch
and adds b_proj.

Everything the PE contracts over lives partition-major: x is fed as xT[c,t];
weights are fed pre-transposed. The whole kernel is a single software
pipeline over t-chunks of 512: qkv(tc) -> attention(tc) -> proj(tc), so the
scalar engine's exp stream overlaps the tensor engine's qkv/proj matmuls.

Attention computes S^T[s,t] = k^T.T @ q^T directly (no transposes in the
S/P path), exp is applied unnormalized (scores are O(1) here), and V is
augmented with 64 ones-columns so the PV matmul yields the softmax
denominator replicated across partitions 64..127 -- normalization is then
one reciprocal_approx_fast + one DVE multiply per (head, chunk). Causality:
above-diagonal s-tiles are skipped; diagonal slabs are masked with
precomputed 0/1 masks. QK^T packs two heads in the PE via row tiling (K=64).
V is produced in [m,t] layout like q/k (wide N=512 matmuls) and moved to the
[t,m] layout PV needs via PE transposes of 128x128 blocks.

Attention-path tensors are fp16 (enables fast weight load, halves SBUF);
PSUM accumulation is always fp32; the projection runs in float32r.
"""

import numpy as np
from contextlib import ExitStack

import concourse.bass as bass
import concourse.bacc as bacc
import concourse.tile as tile
import concourse.mybir as mybir
from concourse.bass_utils import run_bass_kernel_spmd

B, T, C, H = 2, 2048, 1024, 16
D = C // H                  # 64, head dim
HPC = 4                     # heads per core
N_CORES = 8
NT = T // 128               # 16 t-tiles / s-tiles of 128
NCT = C // 128              # 8 contraction tiles over C
TCH = T // 512              # 4 t-chunks of 512
SCALE = 1.0 / np.sqrt(np.float32(C))   # 1/32

F32 = mybir.dt.float32
F32R = mybir.dt.float32r
F16 = mybir.dt.float16
EXP = mybir.ActivationFunctionType.Exp

VW = 2 * D                  # 128: per-head block in v_sb = [v_h (64) | ones (64)]

_CACHE = {}


def _build():
    """Build + compile the SPMD program (identical on all 8 cores)."""
    nc = bacc.Bacc("TRN2", target_bir_lowering=False, debug=False)

    xT = nc.dram_tensor("xT", [C, T], F16, kind="ExternalInput")          # x[b].T
    wqkvT = nc.dram_tensor("wqkvT", [C, 3 * HPC * D], F16, kind="ExternalInput")
    wpT = nc.dram_tensor("wpT", [HPC * D, C], F32R, kind="ExternalInput")
    bqkv = nc.dram_tensor("bqkv", [128, 6], F32, kind="ExternalInput")    # per m-tile
    ident = nc.dram_tensor("ident", [128, 128], F16, kind="ExternalInput")
    vones = nc.dram_tensor("vones", [128, NT * HPC * D], F16, kind="ExternalInput")
    mask = nc.dram_tensor("mask", [128, 2048], F16, kind="ExternalInput")  # 4x[128,512]
    y = nc.dram_tensor("y", [T, C], F32, kind="ExternalOutput")

    with tile.TileContext(nc) as tc, ExitStack() as ctx:
        sb = ctx.enter_context(tc.tile_pool(name="persist", bufs=1))

        # ---- persistent SBUF tensors ----
        wqkv_sb = sb.tile([128, NCT * 768], F16, tag="wqkv")       # [c-tile][m 768]
        wp_sb = sb.tile([128, 2 * C], F32R, tag="wp")              # [ci-tile][co 1024]
        bqkv_sb = sb.tile([128, 6], F32, tag="bqkv")
        ident_sb = sb.tile([128, 128], F16, tag="ident")
        mask_sb = sb.tile([128, 2048], F16, tag="mask")
        qk_sb = sb.tile([128, 6 * T], F16, tag="qk")   # q^T|k^T|v^T [m-tile][t]
        v_sb = sb.tile([128, NT * HPC * VW], F16, tag="v")  # [s-tile][h][v|ones]
        on_sb = sb.tile([128, 2 * T], F32R, tag="onorm")    # O_norm^T [ci-tile][t]

        nc.sync.dma_start(bqkv_sb[:], bqkv.ap())
        nc.sync.dma_start(ident_sb[:], ident.ap())
        for kt in range(2):
            nc.sync.dma_start(wp_sb[:, kt * C:(kt + 1) * C], wpT.ap()[kt * 128:(kt + 1) * 128, :])
        nc.sync.dma_start(mask_sb[:], mask.ap())
        # ones columns of v_sb (softmax denominator trick), cols 64..127/head
        vdst = v_sb[:].rearrange("p (s h e) -> p s h e", s=NT, h=HPC)[:, :, :, D:VW]
        vsrc = vones.ap().rearrange("p (s h e) -> p s h e", s=NT, h=HPC)
        nc.sync.dma_start(vdst, vsrc)

        # ---- fused pipeline: per t-chunk, qkv -> attention -> proj ----
        # PSUM budget (8 banks): sG [128,1024] x2 bufs = 4, acc0+acc1 = 2,
        # shared ps1 pool (qkv accum / v-transpose / proj out) x2 = 2.
        with tc.tile_pool(name="xTp", bufs=1) as xtp, \
             tc.tile_pool(name="ps1", bufs=2, space="PSUM") as ps1, \
             tc.tile_pool(name="ps2", bufs=2, space="PSUM") as ps2, \
             tc.tile_pool(name="psacc", bufs=1, space="PSUM") as psacc, \
             tc.tile_pool(name="att", bufs=4) as att, \
             tc.tile_pool(name="yst", bufs=4) as yst:
            xT_sb = xtp.tile([128, NCT * T], F16, tag="xT")       # [c-tile][t]
            for ct in range(NCT):
                nc.sync.dma_start(wqkv_sb[:, ct * 768:(ct + 1) * 768], wqkvT.ap()[ct * 128:(ct + 1) * 128, :])
                nc.sync.dma_start(xT_sb[:, ct * T:(ct + 1) * T], xT.ap()[ct * 128:(ct + 1) * 128, :])

            for tch in range(TCH):
                # qkv for this t-chunk: m-tiles 0,1=q 2,3=k 4,5=v (4 heads ea)
                for mt in range(6):
                    acc = ps1.tile([128, 512], F32, tag="qkacc")
                    for ct in range(NCT):
                        nc.tensor.matmul(
                            acc[:],
                            wqkv_sb[:, ct * 768 + mt * 128: ct * 768 + (mt + 1) * 128],
                            xT_sb[:, ct * T + tch * 512: ct * T + tch * 512 + 512],
                            start=(ct == 0), stop=(ct == NCT - 1),
                        )
                    nc.vector.tensor_scalar_add(
                        qk_sb[:, mt * T + tch * 512: mt * T + tch * 512 + 512],
                        acc[:], bqkv_sb[:, mt:mt + 1],
                    )
                # v -> [t, m] layout: PE transposes of 128x128 (2 heads/blk)
                for st in range(4 * tch, 4 * tch + 4):
                    for hv in range(2):
                        tp = ps1.tile([128, 512], F32, tag="qkacc")
                        tp16 = tp[:].bitcast(F16)[:, 0:128]
                        nc.tensor.transpose(
                            tp16,
                            qk_sb[:, (4 + hv) * T + st * 128: (4 + hv) * T + st * 128 + 128],
                            ident_sb[:])
                        dst = v_sb[:, st * HPC * VW + 2 * hv * VW: st * HPC * VW + (2 * hv + 2) * VW].rearrange(
                            "p (h e) -> p h e", h=2)[:, :, 0:D]
                        src = tp16.rearrange("p (h d) -> p h d", h=2)
                        nc.vector.tensor_copy(dst, src)

                # attention for this t-chunk
                for hp in range(2):      # head pair (heads 2hp, 2hp+1)
                    qoff = hp * T        # q m-tile = hp
                    koff = (2 + hp) * T  # k m-tile = 2+hp
                    acc0 = psacc.tile([128, 512], F32, tag="acc0")
                    acc1 = psacc.tile([128, 512], F32, tag="acc1")
                    n_slab = 2 * (tch + 1)
                    for g in range(n_slab):
                        sG0 = ps2.tile([128, 1024], F32, tag="sG")
                        sG1 = ps2.tile([128, 1024], F32, tag="sG")
                        for j in range(2):
                            st = 2 * g + j
                            nc.tensor.matmul(
                                sG0[:, j * 512:(j + 1) * 512],
                                qk_sb[0:64, koff + st * 128: koff + st * 128 + 128],
                                qk_sb[0:64, qoff + tch * 512: qoff + tch * 512 + 512],
                                start=True, stop=True, tile_position=(0, 0),
                            )
                            nc.tensor.matmul(
                                sG1[:, j * 512:(j + 1) * 512],
                                qk_sb[64:128, koff + st * 128: koff + st * 128 + 128],
                                qk_sb[64:128, qoff + tch * 512: qoff + tch * 512 + 512],
                                start=True, stop=True, tile_position=(64, 0),
                            )
                        p0 = att.tile([128, 1024], F16, tag="p0")
                        p1 = att.tile([128, 1024], F16, tag="p1")
                        nc.scalar.activation(p0[:], sG0[:], EXP, scale=float(SCALE))
                        nc.scalar.activation(p1[:], sG1[:], EXP, scale=float(SCALE))
                        if g >= 2 * tch:   # diagonal slab: causal 0/1 mask
                            mi = (g - 2 * tch) * 1024
                            m = mask_sb[:, mi:mi + 1024]
                            nc.vector.tensor_mul(p0[:], p0[:], m)
                            nc.vector.tensor_mul(p1[:], p1[:], m)
                        first, last = (g == 0), (g == n_slab - 1)
                        for j in range(2):
                            st = 2 * g + j
                            nc.tensor.matmul(
                                acc0[:],
                                v_sb[:, st * HPC * VW + (2 * hp) * VW: st * HPC * VW + (2 * hp) * VW + VW],
                                p0[:, j * 512:(j + 1) * 512],
                                start=(first and j == 0), stop=(last and j == 1),
                            )
                            nc.tensor.matmul(
                                acc1[:],
                                v_sb[:, st * HPC * VW + (2 * hp + 1) * VW: st * HPC * VW + (2 * hp + 1) * VW + VW],
                                p1[:, j * 512:(j + 1) * 512],
                                start=(first and j == 0), stop=(last and j == 1),
                            )
                    # normalize: O_norm^T = O^T*(1/l), l on rows 64..127
                    for i, acc in ((0, acc0), (1, acc1)):
                        a = 2 * hp + i   # head index in core
                        # full-tile recip: the custom-DVE op mishandles
                        # partition slices; rows 0..63 are garbage, unused
                        rl = att.tile([128, 512], F32, tag="rl")
                        nc.vector.reciprocal_approx_fast(rl[:], acc[:])
                        po = (a % 2) * 64
                        dst = on_sb[po:po + 64,
                                    (a // 2) * T + tch * 512:(a // 2) * T + tch * 512 + 512]
                        nc.vector.tensor_mul(dst, acc[0:D, :], rl[64:128, :])

                # proj for this t-chunk (needs all 4 heads at these t)
                for tt in range(4 * tch, 4 * tch + 4):
                    for cc in range(2):
                        acc = ps1.tile([128, 512], F32, tag="qkacc")
                        for kt in range(2):
                            nc.tensor.matmul(
                                acc[:],
                                on_sb[:, kt * T + tt * 128: kt * T + tt * 128 + 128],
                                wp_sb[:, kt * C + cc * 512: kt * C + cc * 512 + 512],
                                start=(kt == 0), stop=(kt == 1),
                            )
                        ytile = yst.tile([128, 512], F32, tag="ytile")
                        nc.vector.tensor_copy(ytile[:], acc[:])
                        nc.sync.dma_start(
                            y.ap()[tt * 128:(tt + 1) * 128, cc * 512:(cc + 1) * 512],
                            ytile[:],
                        )

    nc.compile()
    return nc


def _causal_masks():
    """mask[p, r*512 + j] = 1.0 if (128*r + p) <= j else 0.0, r in 0..3."""
    p = np.arange(128)[:, None]
    j = np.arange(512)[None, :]
    cols = [((128 * r + p) <= j).astype(np.float32) for r in range(4)]
    return np.concatenate(cols, axis=1)


def _in_maps(x, w_qkv, b_qkv, w_proj):
    mask = _causal_masks()
    vones = np.ones((128, NT * HPC * D), dtype=np.float32)
    maps = []
    for core in range(N_CORES):
        b, hg = divmod(core, 4)
        h0 = hg * HPC                       # first global head of this core
        r0 = h0 * D                         # first q row
        q_w = w_qkv[r0:r0 + HPC * D]                    # [256, C]
        k_w = w_qkv[C + r0:C + r0 + HPC * D]
        v_w = w_qkv[2 * C + r0:2 * C + r0 + HPC * D]
        wqkvT = np.ascontiguousarray(np.concatenate([q_w, k_w, v_w], axis=0).T)
        wpT = np.ascontiguousarray(w_proj[:, r0:r0 + HPC * D].T)    # [256, C]
        bqkv = np.ascontiguousarray(np.concatenate(
            [b_qkv[r0:r0 + HPC * D], b_qkv[C + r0:C + r0 + HPC * D],
             b_qkv[2 * C + r0:2 * C + r0 + HPC * D]]).reshape(6, 128).T)  # [128,6]
        maps.append({
            "xT": np.ascontiguousarray(x[b].T).astype(np.float16),
            "wqkvT": wqkvT.astype(np.float16),
            "wpT": wpT,
            "bqkv": bqkv,
            "ident": np.eye(128, dtype=np.float16),
            "vones": vones.astype(np.float16),
            "mask": mask.astype(np.float16),
        })
    return maps


def kernel(x, w_qkv, b_qkv, w_proj, b_proj, _trace=False, _tmpdir=None):
    x = np.asarray(x, dtype=np.float32)
    w_qkv = np.asarray(w_qkv, dtype=np.float32)
    b_qkv = np.asarray(b_qkv, dtype=np.float32)
    w_proj = np.asarray(w_proj, dtype=np.float32)
    b_proj = np.asarray(b_proj, dtype=np.float32)

    if "nc" not in _CACHE:
        _CACHE["nc"] = _build()
    nc = _CACHE["nc"]

    maps = _in_maps(x, w_qkv, b_qkv, w_proj)
    kw = {}
    if _trace:
        kw = {"trace": True, "tmpdir": _tmpdir}
    res = run_bass_kernel_spmd(nc, maps, list(range(N_CORES)), **kw)

    out = np.empty((B, T, C), dtype=np.float32)
    for b in range(B):
        acc = res.results[4 * b]["y"].astype(np.float32)
        for hg in range(1, 4):
            acc = acc + res.results[4 * b + hg]["y"]
        out[b] = acc + b_proj[None, :]
    if _trace:
        return out, res
    return out
